# revision 54
# baseline (speedup 1.0000x reference)
"""Batch-parallel attention kernel for 8 TRN2 NeuronCores.

Problem: q,k,v [32, 2048, 128] f32 -> out = softmax(q@k^T/sqrt(128)) @ v.

Sharding: batch dim across 8 cores (4 batches/core), no cross-core comm.

Per-core algorithm (per batch, N=2048, D=128); ScalarE exp is the pacing
engine (16.8M exps at 1 elem/cycle/lane @1.2GHz = 109.2us floor):
  - Scores S^T[k, q] per 512-col q-chunk on PE (K^T tile stationary,
    Q^T chunk streaming, f32 PSUM), exp'd by ScalarE in EIGHT
    activations per chunk (2 k-tiles each, 1024 f32 = 2 PSUM banks,
    triple-buffered = 6 banks + 2 MM2-acc banks = the full 8-bank
    PSUM).  Wider groups (fewer ACTIVATEs) measure WORSE: exp duration
    (~1.54us for 1536 elems) exceeds its issue period (~1.42us), so
    with only double-buffered score banks the next-next MM1 gates on
    an exp END and the PE falls off pace; triple-buffering with 2-bank
    tiles keeps every MM1 two windows ahead of its consumer.
  - V_aug [k, t, D+1] with a ones column makes the softmax denominator
    fall out of MM2 (column 128) -- no cross-partition reduction.
  - MM2: out[q,129] accumulated over 16 k-tiles with P^T[k,q] tiles
    stationary (FWL keeps the ~59ns/matmul cadence), then VectorE
    reciprocal+scale.  MM2 work is sliced into 4-matmul QUARTERS, two
    popped per exp window (16/chunk = 4 output blocks), emitted after
    the window's MM1s so they never delay them.
  - Batch 0 is produced k-major: q-chunks 0/1 interleaved (qc1 one
    group behind qc0) so each PE K^T-transpose feeds TWO exps; all
    batch-0 tiles go f32-staging-DMA -> DVE cast -> PE transpose,
    hand-placed in phase-A slots with >=1-group leads.  Staging DMAs
    are split critical-first; they are 512B-line-granular and fair-
    share the DMA engines with the SWDGE floods (big coalesced reads
    would starve them -- measured).
  - Batches 1-3 use SWDGE f32->bf16 cast-DMA (DRAM->DRAM, exact [N,D]
    form to keep line granularity) + xbar transpose-DMA pieces
    (~2.5us per 128 columns), interleaved first-needed-first: all of
    K^T, then Q^T piece by piece.  Batch 1's Q tiles 0-3 come via PE
    transposes dripped into C2/C3 since its qt xbar pieces would land
    after C4.  Batches 2/3 drip 3 chunks ahead of use.
  - No max-subtraction: scores are ~N(0,1), exp is exact to ~2ulp on
    ScalarE and stays in range.

rel_l2 ~3e-3 vs f64 reference (bf16 operand rounding; f32 accumulation).
"""

import math
from collections import deque

import numpy as np

import concourse.bass as bass
import concourse.mybir as mybir
import concourse.tile as tile
from concourse import bacc
from concourse.bass import ts
from concourse.bass_utils import run_bass_kernel_spmd
from concourse.masks import make_identity

B, N, D = 32, 2048, 128
N_CORES = 8
B_LOC = B // N_CORES  # batches per core
NT = N // 128  # 16 k-tiles per batch
QCHUNK = 512
NQC = N // QCHUNK  # 4 q-chunks
NG = 6  # exp groups per chunk: 4x3 + 2x2 k-tiles
# (start k-tile, n k-tiles) per group; 1536-elem ACTIVATEs amortize the
# 172-cycle per-instruction overhead better than 8x1024 (saves ~4.6us of
# ScalarE across the kernel).  3-bank score tiles double-buffered (6
# banks) + 2 accumulator banks = the full 8-bank PSUM; the old "MM1
# gates on exp END" hazard of double-buffering is defused by emitting
# MM1s at high scheduler priority.
GROUPS = [(0, 3), (3, 3), (6, 3), (9, 3), (12, 2), (14, 2)]
SCALE = 1.0 / math.sqrt(D)
FP32 = mybir.dt.float32
BF16 = mybir.dt.bfloat16

_CACHE = {}


def build_nc():
    nc = bacc.Bacc(None, target_bir_lowering=False)
    q_d = nc.dram_tensor("q", [B_LOC, N, D], FP32, kind="ExternalInput")
    k_d = nc.dram_tensor("k", [B_LOC, N, D], FP32, kind="ExternalInput")
    v_d = nc.dram_tensor("v", [B_LOC, N, D], FP32, kind="ExternalInput")
    o_d = nc.dram_tensor("out", [B_LOC, N, D], FP32, kind="ExternalOutput")

    with tile.TileContext(nc) as tc:
        with (
            tc.tile_pool(name="const", bufs=1) as constp,
            tc.tile_pool(name="dram", bufs=3, space="DRAM") as dramp,
            tc.tile_pool(name="stg", bufs=7) as stg,
            tc.tile_pool(name="b16", bufs=4) as b16p,
            tc.tile_pool(name="big", bufs=2) as big,
            tc.tile_pool(name="pt", bufs=4) as ptp,
            tc.tile_pool(name="outp", bufs=6) as outp,
            tc.tile_pool(name="small", bufs=8) as smallp,
            tc.tile_pool(name="st", bufs=2, space="PSUM") as stp,
            tc.tile_pool(name="acc", bufs=2, space="PSUM") as accp,
        ):
            ident = constp.tile([128, 128], BF16)

            batch_tiles = {}

            # -------- batch-0 (and b1-qt0-3) PE-transpose path ---------
            b0 = {}

            def b0_load(key, src_d, b, t0, nt_):
                s = stg.tile([128, nt_, 128], FP32, tag="stg",
                             name=f"s_{key}_{t0}")
                nc.sync.dma_start(
                    s[:],
                    src_d[b, bass.ds(t0 * 128, nt_ * 128), :].rearrange(
                        "(t p) d -> p t d", p=128
                    ),
                )
                b0[("f32", key, t0)] = s

            def b0_cast(key, t0):
                s = b0.pop(("f32", key, t0))
                n = s.shape[1]
                c = b16p.tile([128, n, 128], BF16, tag="b16",
                              name=f"c_{key}_{t0}")
                nc.vector.tensor_copy(c[:], s[:])
                for i in range(n):
                    b0[("b16", key, t0 + i)] = (c, i)

            def b0_tpose(key, t, dst=None):
                c, i = b0[("b16", key, t)]
                t_s = dst if dst is not None else b0["T", key]
                ps = accp.tile([128, 128], BF16, tag="acc")
                nc.tensor.transpose(ps[:], c[:, i, :], ident[:])
                nc.vector.tensor_copy(t_s[:, ts(t, 128)], ps[:])

            # ------------- steady batches: DMA-only setup --------------
            def setup_cast(b, src_d, key):
                # SWDGE cast f32->bf16 DRAM->DRAM, FLAT: one coalesced
                # run lowers to 32KB packets (vs 512B lines), ~6x faster.
                # Safe only because every cast is deferred behind the b0
                # staging loads (coalesced packets would starve them).
                scratch = dramp.tile(
                    [N, D], BF16, tag=key + "d", name=f"sc_{key}_{b}"
                )
                nc.gpsimd.dma_start(
                    scratch[:].rearrange("n d -> (n d)"),
                    src_d[b].rearrange("n d -> (n d)"),
                )
                t_s = big.tile([128, N], BF16, tag=key, name=f"ts_{key}_{b}")
                return scratch, t_s

            def setup_xbar(scratch, t_s, c0, c1):
                # xbar transpose-DMA piece: rows [c0:c1) of the scratch
                # become columns [c0:c1) of the K^T/Q^T tile.
                nc.sync.dma_start(
                    t_s[:, c0:c1], scratch[c0:c1, :], transpose=True
                )

            def setup_load_v(b):
                va = big.tile([128, NT, D + 1], BF16, tag="va",
                              name=f"va_{b}")
                nc.gpsimd.dma_start(
                    va[:, :, 0:D],
                    v_d[b].rearrange("(t p) d -> p t d", p=128),
                )
                nc.vector.memset(va[:, :, D : D + 1], 1.0)
                return va

            vstg_tiles = {}

            def setup_load_v_fast(b):
                # va for b0/b1 via f32 staging (fast HW-queue DMA) + DVE
                # cast -- off the congested SWDGE path, ready ~17/28us.
                vstg = big.tile([128, NT, D], FP32, tag="vstg",
                                name=f"vstg_{b}")
                vstg_tiles[b] = vstg
                nc.sync.dma_start(
                    vstg[:], v_d[b].rearrange("(t p) d -> p t d", p=128)
                )
                va = big.tile([128, NT, D + 1], BF16, tag="va",
                              name=f"va_{b}")
                nc.vector.tensor_copy(va[:, :, 0:D], vstg[:])
                nc.vector.memset(va[:, :, D : D + 1], 1.0)
                return va

            # b2/b3 SWDGE casts are issued EARLY (right after b1's, see
            # below) into their own DRAM scratch (dramp bufs=3, no WAR),
            # so only the va load + xbar pieces remain to drip.
            precast = {}

            def make_setup_ops(b):
                # kt pieces first (all 16 tiles needed at the batch's
                # first chunk), qt piece 0 next, rest of qt trailing.
                state = dict(precast[b])

                def fin():
                    batch_tiles[b] = (state["qt"], state["kt"], state["va"])

                def load_va():
                    # The va DMA carries the va-ring WAR wait (previous
                    # batch's last MM2 read); a late wait_until keeps it
                    # BEHIND the out-DMAs in the gpsimd queue order so
                    # it cannot head-block them while the WAR is open.
                    with tc.tile_wait_until(0.06 + 0.02 * (b - 2)):
                        state["va"] = setup_load_v(b)

                ops = [load_va]
                for key, c0 in (
                    ("kt", 0), ("qt", 0), ("kt", 512), ("kt", 1024),
                    ("kt", 1536), ("qt", 512), ("qt", 1024), ("qt", 1536),
                ):
                    ops.append(
                        lambda key=key, c0=c0: setup_xbar(
                            state[key + "scr"], state[key], c0, c0 + 512
                        )
                    )
                return ops, fin

            # ---------------- MM2 quarter queue ------------------------
            # Quarters are enqueued EAGERLY where the 2 spare PSUM banks
            # allow: blocks 0/1 of a chunk accumulate during the chunk
            # itself (quarter j enqueued right after exp groups 2j/2j+1),
            # blocks 2/3 at chunk end.  Pops stay 2/window (+1 extra on 3
            # windows while the queue is deep).  This keeps the queue
            # shallow, so out-DMAs fire one chunk after their chunk
            # instead of three (the old 32-quarter phase-A backlog echoed
            # through ot-buffer WARs into an 8.9us exp stall at C4).
            # NOTE: one accumulation group per PSUM bank -- packing two
            # concurrent accumulators into one bank corrupts results
            # (bank-level accumulation-group state).
            quarter_q = deque()

            def emit_quarter(job):
                b, qc, qi, quarter, ptile, va, ot_all, meta = job
                if quarter == 0:
                    meta[f"o_ps{qi}"] = accp.tile(
                        [128, D + 1], FP32, tag="acc", name="o_ps"
                    )
                o_ps = meta[f"o_ps{qi}"]
                for kt in range(4 * quarter, 4 * quarter + 4):
                    nc.tensor.matmul(
                        o_ps[:],
                        ptile[:, kt, ts(qi, 128)],
                        va[:, kt, :],
                        start=(kt == 0),
                        stop=(kt == NT - 1),
                    )
                if quarter == 3:
                    rec = smallp.tile([128, 1], FP32)
                    nc.vector.reciprocal(rec[:], o_ps[:, D : D + 1])
                    nc.vector.tensor_scalar_mul(
                        ot_all[:, qi, :], o_ps[:, 0:D], rec[:]
                    )
                    meta["done"] += 1
                    if meta["done"] == NQC:
                        # out-DMAs issue from the (mostly idle) GpSimd
                        # queue -- on Sync they serialize behind the
                        # 1.2-1.5us xbar transpose pieces and the late
                        # completions echo into ot-buffer WAR stalls.
                        # The wait_until keeps them AFTER the deferred
                        # casts in the scheduler's gpsimd queue order:
                        # their Vector-gated waits head-block the queue.
                        with tc.tile_wait_until(0.05):
                            nc.gpsimd.dma_start(
                                o_d[b, ts(qc, QCHUNK), :].rearrange(
                                    "(c p) d -> p c d", p=128
                                ),
                                ot_all[:],
                            )

            def pop_quarters(n):
                for _ in range(n):
                    if quarter_q:
                        emit_quarter(quarter_q.popleft())

            def finish_chunk(b, qc, ptile, va, ot_all):
                meta = {"done": 0}
                for qi in range(QCHUNK // 128):
                    for quarter in range(4):
                        quarter_q.append(
                            (b, qc, qi, quarter, ptile, va, ot_all, meta)
                        )

            # pending setup work: (ops, finish, deadline chunk index).
            # Two drip slots per chunk (groups 0 and NG-1).
            pending = []

            def drip(ci, gi):
                if pending:
                    ops, fin, deadline = pending[0]
                    n_slots = max(1, (deadline - ci) * 2 - (1 if gi else 0))
                    take = max(1, -(-len(ops) // n_slots))
                    for op in ops[:take]:
                        op()
                    del ops[:take]
                    if not ops:
                        fin()
                        pending.pop(0)

            def mm1_group(st, kt_s, qt_s, qc, g):
                k0, n = GROUPS[g]
                for j in range(n):
                    nc.tensor.matmul(
                        st[:, j, :],
                        kt_s[:, ts(k0 + j, 128)],
                        qt_s[:, ts(qc, QCHUNK)],
                        start=True,
                        stop=True,
                    )

            def exp_group(st, ptile, g):
                k0, n = GROUPS[g]
                nc.scalar.activation(
                    ptile[:, k0 : k0 + n, :],
                    st[:, 0:n, :],
                    mybir.ActivationFunctionType.Exp,
                    scale=SCALE,
                )

            # ================= batch 0 ramp =============================
            # Staging DMAs split critical-first; first exp gates on only
            # the first two (kt0-3, qt0-3).
            b0_load("kt", k_d, 0, 0, 4)
            b0_load("qt", q_d, 0, 0, 4)
            b0_load("kt", k_d, 0, 4, 5)
            b0_load("qt", q_d, 0, 4, 4)
            b0_load("kt", k_d, 0, 9, 7)
            b0_load("qt", q_d, 0, 8, 8)
            b0_load("b1q", q_d, 1, 0, 4)
            qt8_stg = b0[("f32", "qt", 8)]
            b0["T", "kt"] = big.tile([128, N], BF16, tag="kt", name="ts_kt_0")
            b0["T", "qt"] = big.tile([128, N], BF16, tag="qt", name="ts_qt_0")
            make_identity(nc, ident[:])
            b0_cast("kt", 0)
            for t in range(4):
                b0_tpose("kt", t)
            b0_cast("qt", 0)
            for t in range(4):
                b0_tpose("qt", t)
            # batch 1: K^T fully via xbar pieces (all 16 tiles needed at
            # C4); Q^T pieces 1-3 via xbar after; Q tiles 0-3 via the PE
            # path (dripped into C2/C3 -- the qt xbars would miss C4).
            # The SWDGE flood (cast-DMAs + va loads) must not contend
            # with the b0 staging DMAs -- unthrottled it pushes qt0-3
            # landing from ~11us to ~28us.  The gate is a dummy GpSimd
            # read of the LAST b0 staging tile (real data dep = real
            # time anchor); tile_wait_until orders the doorbells after
            # it in the scheduler's queue order (the Tile scheduler
            # otherwise hoists ready instructions past a blocked one).
            # Queue order after the gate: va0 (needed ~38us), b1 casts
            # (kt piece 1 needed ~50us), then b2/b3 casts (C8+/C12+).
            b0["va"] = setup_load_v_fast(0)
            with tc.tile_wait_until(0.012):
                # Gate on the LAST Sync staging load (va0's f32 stage):
                # the HW queue completes in order, so this covers every
                # ramp-critical staging DMA.  The flat casts' 32KB
                # packets would otherwise starve the fine-grained
                # staging still in flight (qt8-15 measured 8us late).
                gate = smallp.tile([128, 1], FP32)
                nc.gpsimd.tensor_copy(gate[:], vstg_tiles[0][:, 0, 0:1])
                kt1_scr, kt1_ts = setup_cast(1, k_d, "kt")
                qt1_scr, qt1_ts = setup_cast(1, q_d, "qt")
                for b_ in (2, 3):
                    scr_k, ts_k = setup_cast(b_, k_d, "kt")
                    scr_q, ts_q = setup_cast(b_, q_d, "qt")
                    precast[b_] = {
                        "ktscr": scr_k, "kt": ts_k,
                        "qtscr": scr_q, "qt": ts_q,
                    }
                for scr, t_s, c0 in (
                    (kt1_scr, kt1_ts, 0),
                    (kt1_scr, kt1_ts, 512),
                    (kt1_scr, kt1_ts, 1024),
                    (kt1_scr, kt1_ts, 1536),
                    (qt1_scr, qt1_ts, 512),
                    (qt1_scr, qt1_ts, 1024),
                    (qt1_scr, qt1_ts, 1536),
                ):
                    setup_xbar(scr, t_s, c0, c0 + 512)
            batch_tiles[0] = (b0["T", "qt"], b0["T", "kt"], b0["va"])

            # phase-A drip plan (12 slots, 6 groups x 2 chunks): kt tiles
            # for group g land by the end of the slot before (g, qc0);
            # qt4-7 by the end of slot 1 for (g0, qc1) at slot 2.  b1's
            # qt0-3 PE transposes ride the tail slots.
            slot_ops = {
                0: [lambda: b0_cast("qt", 4), lambda: b0_tpose("qt", 4),
                    lambda: b0_cast("kt", 4), lambda: b0_tpose("kt", 4),
                    lambda: b0_tpose("kt", 5)],
                1: [lambda: b0_tpose("qt", 5), lambda: b0_tpose("qt", 6),
                    lambda: b0_tpose("qt", 7)],
                2: [lambda: b0_tpose("kt", 6), lambda: b0_tpose("kt", 7),
                    lambda: b0_tpose("kt", 8)],
                3: [lambda: b0_cast("kt", 9), lambda: b0_tpose("kt", 9),
                    lambda: b0_tpose("kt", 10)],
                4: [lambda: b0_tpose("kt", 11), lambda: b0_cast("qt", 8),
                    lambda: b0_tpose("qt", 8)],
                5: [lambda: b0_tpose("kt", 12), lambda: b0_tpose("kt", 13),
                    lambda: b0_tpose("qt", 9)],
                6: [lambda: b0_tpose("qt", 10), lambda: b0_tpose("qt", 11)],
                7: [lambda: b0_tpose("kt", 14), lambda: b0_tpose("kt", 15),
                    lambda: b0_tpose("qt", 12)],
                8: [lambda: b0_cast("b1q", 0),
                    lambda: b0_tpose("b1q", 0, dst=qt1_ts),
                    lambda: b0_tpose("qt", 13)],
                9: [lambda: b0_tpose("b1q", 1, dst=qt1_ts),
                    lambda: b0_tpose("b1q", 2, dst=qt1_ts),
                    lambda: b0_tpose("qt", 14)],
                10: [lambda: b0_tpose("b1q", 3, dst=qt1_ts),
                     lambda: b0_tpose("qt", 15)],
            }

            qt0, kt0, va0 = batch_tiles[0]
            pt_a = [
                ptp.tile([128, NT, QCHUNK], BF16, tag="pt", name=f"pt{qc}")
                for qc in (0, 1)
            ]
            ot_a = [
                outp.tile([128, QCHUNK // 128, D], FP32, tag="ot", name=f"ot{qc}")
                for qc in (0, 1)
            ]
            # phase A: q-chunks 0 and 1 k-major, qc1 one group behind.
            slots = [(0, 0), (1, 0)]
            for g in range(NG - 2):
                slots += [(g, 1), (g + 2, 0)]
            slots += [(NG - 2, 1), (NG - 1, 1)]
            for si, (g, qc) in enumerate(slots):
                st = stp.tile([128, 3, QCHUNK], FP32, tag="st")
                mm1_group(st, kt0, qt0, qc, g)
                exp_group(st, pt_a[qc], g)
                if si == 6:
                    batch_tiles[1] = (qt1_ts, kt1_ts, setup_load_v_fast(1))
                for op in slot_ops.get(si, ()):
                    op()
            for qc in (0, 1):
                finish_chunk(0, qc, pt_a[qc], va0, ot_a[qc])

            # ================= steady chunks C2..C15 ====================
            chunks = [(0, 2), (0, 3)] + [
                (b, qc) for b in range(1, B_LOC) for qc in range(NQC)
            ]
            for ci, (b, qc) in enumerate(chunks, start=2):
                if qc == 0 and b + 1 in (2, 3):
                    ops, fin = make_setup_ops(b + 1)
                    pending.append((ops, fin, ci + 4))
                qt_s, kt_s, va = batch_tiles[b]
                ptile = ptp.tile([128, NT, QCHUNK], BF16, tag="pt")
                ot_all = outp.tile([128, QCHUNK // 128, D], FP32, tag="ot")
                meta = {"done": 0}
                for gi in range(NG):
                    st = stp.tile([128, 3, QCHUNK], FP32, tag="st")
                    # MM1s first in each window, emitted at high priority
                    # so the scheduler keeps them ahead of backlog pops
                    # in the PE stream (a pop stalled on the acc-WAR
                    # recip chain must not head-block the next MM1; the
                    # st double-buffer bounds how far MM1s can hoist).
                    with tc.high_priority(offset=150):
                        mm1_group(st, kt_s, qt_s, qc, gi)
                    # quarter j of blocks 0/1 is ready once its k-tiles
                    # are exp'd: q0 (kt0-3) after group 1, q1 (kt4-7)
                    # after group 2, q2 (kt8-11) after group 3.  Blocks
                    # 2/3 go at chunk end (only 2 spare PSUM banks).
                    if gi in (2, 3, 4):
                        j = gi - 2
                        for qi in (0, 1):
                            quarter_q.append(
                                (b, qc, qi, j, ptile, va, ot_all, meta)
                            )
                    pop_quarters(
                        (3 if gi in (1, 2, 3, 4) else 2)
                        + (
                            1
                            if (gi in (2, 4) or ci >= 14)
                            and len(quarter_q) > 2
                            else 0
                        )
                    )
                    if gi in (0, NG - 1):
                        drip(ci, gi)
                    exp_group(st, ptile, gi)
                for qi in (0, 1):
                    quarter_q.append((b, qc, qi, 3, ptile, va, ot_all, meta))
                for qi in (2, 3):
                    for j in range(4):
                        quarter_q.append((b, qc, qi, j, ptile, va, ot_all, meta))

            # drain remaining MM2 quarters
            pop_quarters(len(quarter_q))

    nc.compile()
    return nc


def _get_nc():
    if "nc" not in _CACHE:
        _CACHE["nc"] = build_nc()
    return _CACHE["nc"]


def run(q, k, v, **spmd_kwargs):
    """Run on all 8 cores; returns (full_output, BassKernelResults)."""
    nc = _get_nc()
    q = np.ascontiguousarray(q, dtype=np.float32)
    k = np.ascontiguousarray(k, dtype=np.float32)
    v = np.ascontiguousarray(v, dtype=np.float32)
    in_maps = [
        {
            "q": np.ascontiguousarray(q[i * B_LOC : (i + 1) * B_LOC]),
            "k": np.ascontiguousarray(k[i * B_LOC : (i + 1) * B_LOC]),
            "v": np.ascontiguousarray(v[i * B_LOC : (i + 1) * B_LOC]),
        }
        for i in range(N_CORES)
    ]
    res = run_bass_kernel_spmd(nc, in_maps, core_ids=list(range(N_CORES)), **spmd_kwargs)
    out = np.concatenate([r["out"] for r in res.results], axis=0)
    return out, res


def kernel(q, k, v):
    out, _ = run(q, k, v)
    return out



# revision 55
# speedup vs baseline: 1.0064x; 1.0064x over previous
"""Batch-parallel attention kernel for 8 TRN2 NeuronCores.

Problem: q,k,v [32, 2048, 128] f32 -> out = softmax(q@k^T/sqrt(128)) @ v.

Sharding: batch dim across 8 cores (4 batches/core), no cross-core comm.

Per-core algorithm (per batch, N=2048, D=128); ScalarE exp is the pacing
engine (16.8M exps at 1 elem/cycle/lane @1.2GHz = 109.2us floor):
  - Scores S^T[k, q] per 512-col q-chunk on PE (K^T tile stationary,
    Q^T chunk streaming, f32 PSUM), exp'd by ScalarE in EIGHT
    activations per chunk (2 k-tiles each, 1024 f32 = 2 PSUM banks,
    triple-buffered = 6 banks + 2 MM2-acc banks = the full 8-bank
    PSUM).  Wider groups (fewer ACTIVATEs) measure WORSE: exp duration
    (~1.54us for 1536 elems) exceeds its issue period (~1.42us), so
    with only double-buffered score banks the next-next MM1 gates on
    an exp END and the PE falls off pace; triple-buffering with 2-bank
    tiles keeps every MM1 two windows ahead of its consumer.
  - V_aug [k, t, D+1] with a ones column makes the softmax denominator
    fall out of MM2 (column 128) -- no cross-partition reduction.
  - MM2: out[q,129] accumulated over 16 k-tiles with P^T[k,q] tiles
    stationary (FWL keeps the ~59ns/matmul cadence), then VectorE
    reciprocal+scale.  MM2 work is sliced into 4-matmul QUARTERS, two
    popped per exp window (16/chunk = 4 output blocks), emitted after
    the window's MM1s so they never delay them.
  - Batch 0 is produced k-major: q-chunks 0/1 interleaved (qc1 one
    group behind qc0) so each PE K^T-transpose feeds TWO exps; all
    batch-0 tiles go f32-staging-DMA -> DVE cast -> PE transpose,
    hand-placed in phase-A slots with >=1-group leads.  Staging DMAs
    are split critical-first; they are 512B-line-granular and fair-
    share the DMA engines with the SWDGE floods (big coalesced reads
    would starve them -- measured).
  - Batches 1-3 use SWDGE f32->bf16 cast-DMA (DRAM->DRAM, exact [N,D]
    form to keep line granularity) + xbar transpose-DMA pieces
    (~2.5us per 128 columns), interleaved first-needed-first: all of
    K^T, then Q^T piece by piece.  Batch 1's Q tiles 0-3 come via PE
    transposes dripped into C2/C3 since its qt xbar pieces would land
    after C4.  Batches 2/3 drip 3 chunks ahead of use.
  - No max-subtraction: scores are ~N(0,1), exp is exact to ~2ulp on
    ScalarE and stays in range.

rel_l2 ~3e-3 vs f64 reference (bf16 operand rounding; f32 accumulation).
"""

import math
from collections import deque

import numpy as np

import concourse.bass as bass
import concourse.mybir as mybir
import concourse.tile as tile
from concourse import bacc
from concourse.bass import ts
from concourse.bass_utils import run_bass_kernel_spmd
from concourse.masks import make_identity

B, N, D = 32, 2048, 128
N_CORES = 8
B_LOC = B // N_CORES  # batches per core
NT = N // 128  # 16 k-tiles per batch
QCHUNK = 512
NQC = N // QCHUNK  # 4 q-chunks
NG = 8  # exp groups per chunk, 2 k-tiles each
SCALE = 1.0 / math.sqrt(D)
FP32 = mybir.dt.float32
BF16 = mybir.dt.bfloat16

_CACHE = {}


def build_nc():
    nc = bacc.Bacc(None, target_bir_lowering=False)
    q_d = nc.dram_tensor("q", [B_LOC, N, D], FP32, kind="ExternalInput")
    k_d = nc.dram_tensor("k", [B_LOC, N, D], FP32, kind="ExternalInput")
    v_d = nc.dram_tensor("v", [B_LOC, N, D], FP32, kind="ExternalInput")
    o_d = nc.dram_tensor("out", [B_LOC, N, D], FP32, kind="ExternalOutput")

    with tile.TileContext(nc) as tc:
        with (
            tc.tile_pool(name="const", bufs=1) as constp,
            tc.tile_pool(name="dram", bufs=3, space="DRAM") as dramp,
            tc.tile_pool(name="stg", bufs=7) as stg,
            tc.tile_pool(name="b16", bufs=4) as b16p,
            tc.tile_pool(name="big", bufs=2) as big,
            tc.tile_pool(name="pt", bufs=4) as ptp,
            tc.tile_pool(name="outp", bufs=6) as outp,
            tc.tile_pool(name="small", bufs=8) as smallp,
            tc.tile_pool(name="st", bufs=3, space="PSUM") as stp,
            tc.tile_pool(name="acc", bufs=2, space="PSUM") as accp,
        ):
            ident = constp.tile([128, 128], BF16)

            batch_tiles = {}

            # -------- batch-0 (and b1-qt0-3) PE-transpose path ---------
            b0 = {}

            def b0_load(key, src_d, b, t0, nt_):
                s = stg.tile([128, nt_, 128], FP32, tag="stg",
                             name=f"s_{key}_{t0}")
                nc.sync.dma_start(
                    s[:],
                    src_d[b, bass.ds(t0 * 128, nt_ * 128), :].rearrange(
                        "(t p) d -> p t d", p=128
                    ),
                )
                b0[("f32", key, t0)] = s

            def b0_cast(key, t0):
                s = b0.pop(("f32", key, t0))
                n = s.shape[1]
                c = b16p.tile([128, n, 128], BF16, tag="b16",
                              name=f"c_{key}_{t0}")
                nc.vector.tensor_copy(c[:], s[:])
                for i in range(n):
                    b0[("b16", key, t0 + i)] = (c, i)

            def b0_tpose(key, t, dst=None):
                c, i = b0[("b16", key, t)]
                t_s = dst if dst is not None else b0["T", key]
                ps = accp.tile([128, 128], BF16, tag="acc")
                nc.tensor.transpose(ps[:], c[:, i, :], ident[:])
                nc.vector.tensor_copy(t_s[:, ts(t, 128)], ps[:])

            # ------------- steady batches: DMA-only setup --------------
            def setup_cast(b, src_d, key):
                # SWDGE cast f32->bf16 DRAM->DRAM, FLAT: one coalesced
                # run lowers to 32KB packets (vs 512B lines), ~6x faster.
                # Safe only because every cast is deferred behind the b0
                # staging loads (coalesced packets would starve them).
                scratch = dramp.tile(
                    [N, D], BF16, tag=key + "d", name=f"sc_{key}_{b}"
                )
                nc.gpsimd.dma_start(
                    scratch[:].rearrange("n d -> (n d)"),
                    src_d[b].rearrange("n d -> (n d)"),
                )
                t_s = big.tile([128, N], BF16, tag=key, name=f"ts_{key}_{b}")
                return scratch, t_s

            def setup_xbar(scratch, t_s, c0, c1):
                # xbar transpose-DMA piece: rows [c0:c1) of the scratch
                # become columns [c0:c1) of the K^T/Q^T tile.
                nc.sync.dma_start(
                    t_s[:, c0:c1], scratch[c0:c1, :], transpose=True
                )

            def setup_load_v(b):
                va = big.tile([128, NT, D + 1], BF16, tag="va",
                              name=f"va_{b}")
                nc.gpsimd.dma_start(
                    va[:, :, 0:D],
                    v_d[b].rearrange("(t p) d -> p t d", p=128),
                )
                nc.vector.memset(va[:, :, D : D + 1], 1.0)
                return va

            vstg_tiles = {}

            def setup_load_v_fast(b):
                # va for b0/b1 via f32 staging (fast HW-queue DMA) + DVE
                # cast -- off the congested SWDGE path, ready ~17/28us.
                vstg = big.tile([128, NT, D], FP32, tag="vstg",
                                name=f"vstg_{b}")
                vstg_tiles[b] = vstg
                nc.sync.dma_start(
                    vstg[:], v_d[b].rearrange("(t p) d -> p t d", p=128)
                )
                va = big.tile([128, NT, D + 1], BF16, tag="va",
                              name=f"va_{b}")
                nc.vector.tensor_copy(va[:, :, 0:D], vstg[:])
                nc.vector.memset(va[:, :, D : D + 1], 1.0)
                return va

            # b2/b3 SWDGE casts are issued EARLY (right after b1's, see
            # below) into their own DRAM scratch (dramp bufs=3, no WAR),
            # so only the va load + xbar pieces remain to drip.
            precast = {}

            def make_setup_ops(b):
                # kt pieces first (all 16 tiles needed at the batch's
                # first chunk), qt piece 0 next, rest of qt trailing.
                state = dict(precast[b])

                def fin():
                    batch_tiles[b] = (state["qt"], state["kt"], state["va"])

                def load_va():
                    # The va DMA carries the va-ring WAR wait (previous
                    # batch's last MM2 read); a late wait_until keeps it
                    # BEHIND the out-DMAs in the gpsimd queue order so
                    # it cannot head-block them while the WAR is open.
                    with tc.tile_wait_until(0.06 + 0.02 * (b - 2)):
                        state["va"] = setup_load_v(b)

                ops = [load_va]
                for key, c0 in (
                    ("kt", 0), ("qt", 0), ("kt", 512), ("kt", 1024),
                    ("kt", 1536), ("qt", 512), ("qt", 1024), ("qt", 1536),
                ):
                    ops.append(
                        lambda key=key, c0=c0: setup_xbar(
                            state[key + "scr"], state[key], c0, c0 + 512
                        )
                    )
                return ops, fin

            # ---------------- MM2 quarter queue ------------------------
            # Quarters are enqueued EAGERLY where the 2 spare PSUM banks
            # allow: blocks 0/1 of a chunk accumulate during the chunk
            # itself (quarter j enqueued right after exp groups 2j/2j+1),
            # blocks 2/3 at chunk end.  Pops stay 2/window (+1 extra on 3
            # windows while the queue is deep).  This keeps the queue
            # shallow, so out-DMAs fire one chunk after their chunk
            # instead of three (the old 32-quarter phase-A backlog echoed
            # through ot-buffer WARs into an 8.9us exp stall at C4).
            # NOTE: one accumulation group per PSUM bank -- packing two
            # concurrent accumulators into one bank corrupts results
            # (bank-level accumulation-group state).
            quarter_q = deque()

            def emit_quarter(job):
                b, qc, qi, quarter, ptile, va, ot_all, meta = job
                if quarter == 0:
                    meta[f"o_ps{qi}"] = accp.tile(
                        [128, D + 1], FP32, tag="acc", name="o_ps"
                    )
                o_ps = meta[f"o_ps{qi}"]
                for kt in range(4 * quarter, 4 * quarter + 4):
                    nc.tensor.matmul(
                        o_ps[:],
                        ptile[:, kt, ts(qi, 128)],
                        va[:, kt, :],
                        start=(kt == 0),
                        stop=(kt == NT - 1),
                    )
                if quarter == 3:
                    rec = smallp.tile([128, 1], FP32)
                    nc.vector.reciprocal(rec[:], o_ps[:, D : D + 1])
                    nc.vector.tensor_scalar_mul(
                        ot_all[:, qi, :], o_ps[:, 0:D], rec[:]
                    )
                    meta["done"] += 1
                    if meta["done"] == NQC:
                        # out-DMAs issue from the (mostly idle) GpSimd
                        # queue -- on Sync they serialize behind the
                        # 1.2-1.5us xbar transpose pieces and the late
                        # completions echo into ot-buffer WAR stalls.
                        # The wait_until keeps them AFTER the deferred
                        # casts in the scheduler's gpsimd queue order:
                        # their Vector-gated waits head-block the queue.
                        with tc.tile_wait_until(0.05):
                            nc.gpsimd.dma_start(
                                o_d[b, ts(qc, QCHUNK), :].rearrange(
                                    "(c p) d -> p c d", p=128
                                ),
                                ot_all[:],
                            )

            def pop_quarters(n):
                for _ in range(n):
                    if quarter_q:
                        emit_quarter(quarter_q.popleft())

            def finish_chunk(b, qc, ptile, va, ot_all):
                meta = {"done": 0}
                for qi in range(QCHUNK // 128):
                    for quarter in range(4):
                        quarter_q.append(
                            (b, qc, qi, quarter, ptile, va, ot_all, meta)
                        )

            # pending setup work: (ops, finish, deadline chunk index).
            # Two drip slots per chunk (groups 0 and NG-1).
            pending = []

            def drip(ci, gi):
                if pending:
                    ops, fin, deadline = pending[0]
                    n_slots = max(1, (deadline - ci) * 2 - (1 if gi else 0))
                    take = max(1, -(-len(ops) // n_slots))
                    for op in ops[:take]:
                        op()
                    del ops[:take]
                    if not ops:
                        fin()
                        pending.pop(0)

            def mm1_group(st, kt_s, qt_s, qc, g):
                for j in range(2):
                    nc.tensor.matmul(
                        st[:, j, :],
                        kt_s[:, ts(2 * g + j, 128)],
                        qt_s[:, ts(qc, QCHUNK)],
                        start=True,
                        stop=True,
                    )

            def exp_group(st, ptile, g):
                nc.scalar.activation(
                    ptile[:, 2 * g : 2 * g + 2, :],
                    st[:],
                    mybir.ActivationFunctionType.Exp,
                    scale=SCALE,
                )

            # ================= batch 0 ramp =============================
            # Staging DMAs split critical-first; first exp gates on only
            # the first two (kt0-3, qt0-3).
            b0_load("kt", k_d, 0, 0, 4)
            b0_load("qt", q_d, 0, 0, 4)
            b0_load("kt", k_d, 0, 4, 5)
            b0_load("qt", q_d, 0, 4, 4)
            b0_load("kt", k_d, 0, 9, 7)
            b0_load("qt", q_d, 0, 8, 8)
            b0_load("b1q", q_d, 1, 0, 4)
            qt8_stg = b0[("f32", "qt", 8)]
            b0["T", "kt"] = big.tile([128, N], BF16, tag="kt", name="ts_kt_0")
            b0["T", "qt"] = big.tile([128, N], BF16, tag="qt", name="ts_qt_0")
            make_identity(nc, ident[:])
            b0_cast("kt", 0)
            for t in range(4):
                b0_tpose("kt", t)
            b0_cast("qt", 0)
            for t in range(4):
                b0_tpose("qt", t)
            # batch 1: K^T fully via xbar pieces (all 16 tiles needed at
            # C4); Q^T pieces 1-3 via xbar after; Q tiles 0-3 via the PE
            # path (dripped into C2/C3 -- the qt xbars would miss C4).
            # The SWDGE flood (cast-DMAs + va loads) must not contend
            # with the b0 staging DMAs -- unthrottled it pushes qt0-3
            # landing from ~11us to ~28us.  The gate is a dummy GpSimd
            # read of the LAST b0 staging tile (real data dep = real
            # time anchor); tile_wait_until orders the doorbells after
            # it in the scheduler's queue order (the Tile scheduler
            # otherwise hoists ready instructions past a blocked one).
            # Queue order after the gate: va0 (needed ~38us), b1 casts
            # (kt piece 1 needed ~50us), then b2/b3 casts (C8+/C12+).
            b0["va"] = setup_load_v_fast(0)
            with tc.tile_wait_until(0.012):
                # Gate on the LAST Sync staging load (va0's f32 stage):
                # the HW queue completes in order, so this covers every
                # ramp-critical staging DMA.  The flat casts' 32KB
                # packets would otherwise starve the fine-grained
                # staging still in flight (qt8-15 measured 8us late).
                gate = smallp.tile([128, 1], FP32)
                nc.gpsimd.tensor_copy(gate[:], vstg_tiles[0][:, 0, 0:1])
                kt1_scr, kt1_ts = setup_cast(1, k_d, "kt")
                qt1_scr, qt1_ts = setup_cast(1, q_d, "qt")
                for b_ in (2, 3):
                    scr_k, ts_k = setup_cast(b_, k_d, "kt")
                    scr_q, ts_q = setup_cast(b_, q_d, "qt")
                    precast[b_] = {
                        "ktscr": scr_k, "kt": ts_k,
                        "qtscr": scr_q, "qt": ts_q,
                    }
                for scr, t_s, c0 in (
                    (kt1_scr, kt1_ts, 0),
                    (kt1_scr, kt1_ts, 512),
                    (kt1_scr, kt1_ts, 1024),
                    (kt1_scr, kt1_ts, 1536),
                    (qt1_scr, qt1_ts, 512),
                    (qt1_scr, qt1_ts, 1024),
                    (qt1_scr, qt1_ts, 1536),
                ):
                    setup_xbar(scr, t_s, c0, c0 + 512)
            batch_tiles[0] = (b0["T", "qt"], b0["T", "kt"], b0["va"])

            # phase-A drip plan: 2 PE transposes max per slot (casts are
            # DVE-only).  kt pair for group g is ready >=2 slots before
            # its (g, qc0) slot; qt4-7 lands just before (g0, qc1).
            slot_ops = {
                0: [lambda: b0_cast("qt", 4), lambda: b0_tpose("qt", 4),
                    lambda: b0_tpose("qt", 5)],
                1: [lambda: b0_tpose("qt", 6), lambda: b0_tpose("qt", 7)],
                2: [lambda: b0_cast("kt", 4), lambda: b0_tpose("kt", 4),
                    lambda: b0_tpose("kt", 5)],
                3: [lambda: b0_tpose("kt", 6), lambda: b0_tpose("kt", 7)],
                4: [lambda: b0_tpose("kt", 8)],
                5: [lambda: b0_cast("kt", 9), lambda: b0_tpose("kt", 9)],
                6: [lambda: b0_tpose("kt", 10), lambda: b0_tpose("kt", 11)],
                7: [lambda: b0_tpose("kt", 12)],
                8: [lambda: b0_tpose("kt", 13)],
                9: [lambda: b0_tpose("kt", 14)],
                10: [lambda: b0_tpose("kt", 15)],
                11: [lambda: b0_cast("qt", 8), lambda: b0_tpose("qt", 8)],
                12: [lambda: b0_tpose("qt", 9), lambda: b0_tpose("qt", 10)],
                13: [lambda: b0_tpose("qt", 11), lambda: b0_tpose("qt", 12)],
                14: [lambda: b0_tpose("qt", 13), lambda: b0_tpose("qt", 14)],
                15: [lambda: b0_tpose("qt", 15)],
            }
            leftover = [
                lambda: b0_cast("b1q", 0),
                lambda: b0_tpose("b1q", 0, dst=qt1_ts),
                lambda: b0_tpose("b1q", 1, dst=qt1_ts),
                lambda: b0_tpose("b1q", 2, dst=qt1_ts),
                lambda: b0_tpose("b1q", 3, dst=qt1_ts),
            ]

            qt0, kt0, va0 = batch_tiles[0]
            pt_a = [
                ptp.tile([128, NT, QCHUNK], BF16, tag="pt", name=f"pt{qc}")
                for qc in (0, 1)
            ]
            ot_a = [
                outp.tile([128, QCHUNK // 128, D], FP32, tag="ot", name=f"ot{qc}")
                for qc in (0, 1)
            ]
            # phase A: q-chunks 0 and 1 k-major, qc1 one group behind.
            slots = [(0, 0), (1, 0)]
            for g in range(NG - 2):
                slots += [(g, 1), (g + 2, 0)]
            slots += [(NG - 2, 1), (NG - 1, 1)]
            for si, (g, qc) in enumerate(slots):
                st = stp.tile([128, 2, QCHUNK], FP32, tag="st")
                mm1_group(st, kt0, qt0, qc, g)
                exp_group(st, pt_a[qc], g)
                if si == 8:
                    batch_tiles[1] = (qt1_ts, kt1_ts, setup_load_v_fast(1))
                for op in slot_ops.get(si, ()):
                    op()
            for qc in (0, 1):
                finish_chunk(0, qc, pt_a[qc], va0, ot_a[qc])
            pending.append((leftover, lambda: None, 4))

            # ================= steady chunks C2..C15 ====================
            chunks = [(0, 2), (0, 3)] + [
                (b, qc) for b in range(1, B_LOC) for qc in range(NQC)
            ]
            for ci, (b, qc) in enumerate(chunks, start=2):
                if qc == 0 and b + 1 in (2, 3):
                    ops, fin = make_setup_ops(b + 1)
                    pending.append((ops, fin, ci + 4))
                qt_s, kt_s, va = batch_tiles[b]
                ptile = ptp.tile([128, NT, QCHUNK], BF16, tag="pt")
                ot_all = outp.tile([128, QCHUNK // 128, D], FP32, tag="ot")
                meta = {"done": 0}
                for gi in range(NG):
                    st = stp.tile([128, 2, QCHUNK], FP32, tag="st")
                    # MM1s first in each window, emitted at high priority
                    # so the scheduler keeps them ahead of backlog pops
                    # in the PE stream (a pop stalled on the acc-WAR
                    # recip chain must not head-block the next MM1; the
                    # st triple-buffer bounds how far MM1s can hoist).
                    with tc.high_priority(offset=150):
                        mm1_group(st, kt_s, qt_s, qc, gi)
                    # quarter j of blocks 0/1 is ready once exp groups
                    # 2j/2j+1 are emitted (window 2j+1) -- enqueue at
                    # window 2j+2.  Blocks 2/3 go at chunk end (only 2
                    # spare PSUM banks for accumulators).
                    if gi >= 2 and gi % 2 == 0:
                        j = (gi - 2) // 2
                        for qi in (0, 1):
                            quarter_q.append(
                                (b, qc, qi, j, ptile, va, ot_all, meta)
                            )
                    pop_quarters(
                        2
                        + (
                            1
                            if (gi in (1, 3, 5) or ci >= 14)
                            and len(quarter_q) > 2
                            else 0
                        )
                    )
                    if gi in (0, NG - 1):
                        drip(ci, gi)
                    exp_group(st, ptile, gi)
                for qi in (0, 1):
                    quarter_q.append((b, qc, qi, 3, ptile, va, ot_all, meta))
                for qi in (2, 3):
                    for j in range(4):
                        quarter_q.append((b, qc, qi, j, ptile, va, ot_all, meta))

            # drain remaining MM2 quarters
            pop_quarters(len(quarter_q))

    nc.compile()
    return nc


def _get_nc():
    if "nc" not in _CACHE:
        _CACHE["nc"] = build_nc()
    return _CACHE["nc"]


def run(q, k, v, **spmd_kwargs):
    """Run on all 8 cores; returns (full_output, BassKernelResults)."""
    nc = _get_nc()
    q = np.ascontiguousarray(q, dtype=np.float32)
    k = np.ascontiguousarray(k, dtype=np.float32)
    v = np.ascontiguousarray(v, dtype=np.float32)
    in_maps = [
        {
            "q": np.ascontiguousarray(q[i * B_LOC : (i + 1) * B_LOC]),
            "k": np.ascontiguousarray(k[i * B_LOC : (i + 1) * B_LOC]),
            "v": np.ascontiguousarray(v[i * B_LOC : (i + 1) * B_LOC]),
        }
        for i in range(N_CORES)
    ]
    res = run_bass_kernel_spmd(nc, in_maps, core_ids=list(range(N_CORES)), **spmd_kwargs)
    out = np.concatenate([r["out"] for r in res.results], axis=0)
    return out, res


def kernel(q, k, v):
    out, _ = run(q, k, v)
    return out

